# revision 5
# baseline (speedup 1.0000x reference)
"""Trainium2 Bass kernel for nn_CrossNetwork (3x [BatchNorm1d -> cross update]).

Math per layer (reference):
    mu   = mean(x, axis=0)                  # over batch B
    var  = mean((x-mu)^2, axis=0)           # biased
    xn   = (x - mu) * gamma/sqrt(var+eps) + beta
    s    = xn @ w                           # per-row dot over features L
    x'   = x0 * s[:, None] + b + xn

Sharding: L-shard (feature/model parallel). Each of the 8 cores owns 128
of the 1024 features, laid out transposed in SBUF as [128 feature
partitions, 16384 batch free]. BatchNorm stats are core-local free-dim
reductions; the only cross-core exchange is the AllReduce of the
per-core partial dot s (fp16, one per half per layer).

Restructure: xn is never materialized. With g = gamma*rsqrt(var+eps)
and c = beta - mu*g, per layer
    x_{k+1} = g_k x_k + (c_k + b_k) + x0 * s_k .
The per-partition constant is deferred into an offset D (D_0 = 0,
D_{k+1} = g_k D_k + c_k + b_k); the stored tensor follows
    X_{k+1} = g_k X_k + t_k,   t_k = x0 * s_k,   true x_k = X_k + D_k.
The dot reads raw X_k with folded weights (w g W) and each core adds
its batch-independent share sum_p w (g D + c) at PSUM drain, pre-
AllReduce (the AllReduce then sums shares to the global constant).
Stats ride accumulators: Sum(X') on the fused update
(scalar_tensor_tensor), Sum(X'^2) on the ACT Square pass;
mean_true = Sum(X')/B + D, var = Sum(X'^2)/B - (Sum(X')/B)^2.
The last layer's ACT output pass applies +D_3 and writes f32.

Bulk data is fp16 (tolerance 2e-2, measured ~1.4e-3): DVE tensor_scalar
runs 4x, tensor_tensor 2x. The all-reduced s is partition-broadcast
into SBUF by DMA (stride-0 source AP) so the t-pass stays off PSUM.
The g-scale pass (X = g*X) is hoisted into the AllReduce shadow, and a
dummy warmup AllReduce at kernel start absorbs the ~45us communicator
init that otherwise lands on layer 1's critical path.
"""

import os
import sys

import numpy as np

for _p in ("/opt/trn_rl_repo", "/root/.axon_site/_ro/trn_rl_repo"):
    if os.path.isdir(_p) and _p not in sys.path:
        sys.path.insert(0, _p)
        break

P = 128          # feature partitions per core
NCORES = 8
NL = 3
EPS = 1e-8
MAGIC = 0x5F3759DF
WSCALE = 4096.0   # keeps w (~1e-4) out of fp16 subnormal range in the lhs


def build_nc(F=16384, n_cores=NCORES, debug=False):
    """Builds + returns the Bacc module (uncompiled). F = batch per core."""
    from contextlib import ExitStack

    import concourse.bacc as bacc
    import concourse.bass_isa as bass_isa
    import concourse.mybir as mybir
    import concourse.tile as tile
    from concourse.alu_op_type import AluOpType as alu

    dt = mybir.dt
    f32 = dt.float32
    fp16 = dt.float16
    i32 = dt.int32
    AF = mybir.ActivationFunctionType
    AX = mybir.AxisListType

    CH = 2048                 # working chunk (free dim)
    NCH = F // CH             # chunks per tensor
    HALF = F // 2
    HCH = NCH // 2            # chunks per half
    invW = 1.0 / WSCALE

    nc = bacc.Bacc("TRN2", target_bir_lowering=False, debug=debug,
                   num_devices=n_cores)

    xT = nc.dram_tensor("xT", [P, F], f32, kind="ExternalInput").ap()
    par = nc.dram_tensor("par", [P, 12], f32, kind="ExternalInput").ap()
    outT = nc.dram_tensor("outT", [P, F], f32, kind="ExternalOutput").ap()
    cc_in = [nc.dram_tensor(f"cc_in{h}", [HALF], fp16).ap() for h in range(2)]
    cc_out = [nc.dram_tensor(f"cc_out{h}", [HALF], fp16,
                             addr_space="Shared").ap() for h in range(2)]
    ccw_in = nc.dram_tensor("ccw_in", [64], fp16).ap()
    ccw_out = nc.dram_tensor("ccw_out", [64], fp16, addr_space="Shared").ap()
    rg = [list(range(n_cores))]

    with tile.TileContext(nc) as tc, ExitStack() as ctx:
        big = ctx.enter_context(tc.tile_pool(name="big", bufs=1))
        sm = ctx.enter_context(tc.tile_pool(name="small", bufs=1))
        io = ctx.enter_context(tc.tile_pool(name="io", bufs=4))
        tp = ctx.enter_context(tc.tile_pool(name="tmul", bufs=4))
        pdot = ctx.enter_context(tc.tile_pool(name="pdot", bufs=2, space="PSUM"))

        # persistent big tiles, chunked so Tile pipelines at chunk grain
        X0 = [big.tile([P, CH], fp16, tag=f"X0_{i}", name=f"X0_{i}")
              for i in range(NCH)]
        X = [big.tile([P, CH], fp16, tag=f"X_{i}", name=f"X_{i}")
             for i in range(NCH)]
        s_sb = [big.tile([P, HALF], fp16, tag=f"ssb{h}", name=f"ssb{h}")
                for h in range(2)]
        stg = [big.tile([1, HALF], fp16, tag=f"stg{h}", name=f"stg{h}")
               for h in range(2)]
        wrm = big.tile([1, 64], fp16, tag="wrm", name="wrm")
        fdump = big.tile([P, CH], fp16, tag="fdump", name="fdump")

        par_sb = sm.tile([P, 12], f32)
        w_h = sm.tile([P, NL], fp16)
        ts_acc = sm.tile([P, NCH], f32)
        sq_acc = sm.tile([P, NCH], f32)
        geff = sm.tile([P, 1], f32)
        cbias = sm.tile([P, 1], f32)
        Dof = sm.tile([P, 1], f32)
        veps = sm.tile([P, 1], f32)
        rsq = sm.tile([P, 1], f32)
        nt1 = sm.tile([P, 1], f32)
        nt2 = sm.tile([P, 1], f32)
        shrv = sm.tile([P, 1], f32)
        tsum = sm.tile([P, 1], f32)
        ssum = sm.tile([P, 1], f32)
        mean_t = sm.tile([P, 1], f32)
        var_t = sm.tile([P, 1], f32)

        # warmup collective: absorbs communicator init during the load
        nc.vector.memset(wrm[:], 0.0)
        nc.sync.dma_start(ccw_in.rearrange("(o e) -> o e", o=1), wrm[:])
        nc.gpsimd.collective_compute(
            "AllReduce", mybir.AluOpType.add, replica_groups=rg,
            ins=[ccw_in], outs=[ccw_out])

        nc.sync.dma_start(par_sb[:], par[:])
        nc.vector.memset(Dof[:], 0.0)

        # ---- load: DMA f32 chunk -> fp16 convert (DVE, accum Sum x) ----
        # ----       -> Square (ACT, accum Sum x^2)                   ----
        for c in range(NCH):
            sl = slice(c * CH, (c + 1) * CH)
            ld = io.tile([P, CH], f32, tag="io", name="io")
            nc.sync.dma_start(ld[:], xT[:, sl])
            nc.vector.tensor_scalar(X0[c][:], ld[:], 1.0, 0.0,
                                    alu.mult, alu.add,
                                    accum_out=ts_acc[:, c:c + 1])
            nc.scalar.activation(fdump[:], X0[c][:], AF.Square,
                                 accum_out=sq_acc[:, c:c + 1])

        def finalize_stats():
            """mean/var of stored X from the chunk accumulators."""
            nc.vector.tensor_reduce(tsum[:], ts_acc[:], AX.X, alu.add)
            nc.vector.tensor_reduce(ssum[:], sq_acc[:], AX.X, alu.add)
            # m_st = Sum(X)/F ; mean_true = m_st + D ; var = Sum(X^2)/F - m_st^2
            nc.vector.tensor_scalar(tsum[:], tsum[:], 1.0 / F, None, alu.mult)
            nc.vector.tensor_tensor(mean_t[:], tsum[:], Dof[:], alu.add)
            nc.vector.tensor_tensor(nt1[:], tsum[:], tsum[:], alu.mult)
            nc.vector.tensor_scalar(ssum[:], ssum[:], 1.0 / F, None, alu.mult)
            nc.vector.tensor_tensor(var_t[:], ssum[:], nt1[:], alu.subtract)

        finalize_stats()

        def layer_params(k):
            # rsqrt(var+eps): quake seed + 3 Newton iterations (all DVE)
            nc.vector.tensor_scalar(veps[:], var_t[:], EPS, None, alu.add)
            vi = veps[:].bitcast(i32)
            ri = rsq[:].bitcast(i32)
            nc.vector.tensor_scalar(ri, vi, 1, None, alu.logical_shift_right)
            nc.vector.tensor_scalar(ri, ri, -1, MAGIC, alu.mult, alu.add)
            r = rsq[:]
            for _ in range(3):
                nc.vector.tensor_tensor(nt1[:], r, r, alu.mult)
                nc.vector.tensor_tensor(nt1[:], nt1[:], veps[:], alu.mult)
                nc.vector.tensor_scalar(nt1[:], nt1[:], -0.5, 1.5,
                                        alu.mult, alu.add)
                nc.vector.tensor_tensor(r, r, nt1[:], alu.mult)
            # g = gamma * rsqrt ; c = beta - mean_true * g
            nc.vector.tensor_tensor(geff[:], par_sb[:, k:k + 1], r, alu.mult)
            nc.vector.tensor_tensor(nt1[:], mean_t[:], geff[:], alu.mult)
            nc.vector.tensor_tensor(cbias[:], par_sb[:, 3 + k:4 + k], nt1[:],
                                    alu.subtract)
            # share = sum_p w*(g*D + c)  (this core's slice; AR sums them)
            nc.vector.tensor_tensor(nt2[:], geff[:], Dof[:], alu.mult)
            nc.vector.tensor_tensor(nt2[:], nt2[:], cbias[:], alu.add)
            nc.vector.tensor_tensor(nt2[:], nt2[:], par_sb[:, 6 + k:7 + k],
                                    alu.mult)
            nc.gpsimd.partition_all_reduce(shrv[:], nt2[:], P,
                                           bass_isa.ReduceOp.add)
            # folded dot weights: (w*g)*WSCALE, fp16 lhs
            nc.vector.tensor_tensor(nt1[:], par_sb[:, 6 + k:7 + k], geff[:],
                                    alu.mult)
            nc.vector.tensor_scalar(w_h[:, k:k + 1], nt1[:], WSCALE, None,
                                    alu.mult)
            # D' = g*D + (c + b)   (after share, which uses the old D)
            nc.vector.tensor_tensor(nt1[:], cbias[:],
                                    par_sb[:, 9 + k:10 + k], alu.add)
            nc.vector.tensor_scalar(Dof[:], Dof[:], geff[:], nt1[:],
                                    alu.mult, alu.add)

        def half_dots(k, h, src):
            """dots -> drain (+share, /WSCALE, fp16) -> cc DMA -> AllReduce."""
            for cc in range(HCH):
                c = h * HCH + cc
                pd = pdot.tile([1, CH], f32, tag="pd", name="pd")
                for j in range(CH // 512):
                    rhs = src[c][:, j * 512:(j + 1) * 512]
                    nc.tensor.matmul(pd[0:1, j * 512:(j + 1) * 512],
                                     w_h[:, k:k + 1], rhs,
                                     start=True, stop=True)
                dst = stg[h][0:1, cc * CH:(cc + 1) * CH]
                if cc == 0:
                    nc.vector.tensor_scalar(dst, pd[0:1, :], invW,
                                            shrv[0:1, :], alu.mult, alu.add)
                else:
                    nc.scalar.activation(dst, pd[0:1, :], AF.Identity,
                                         bias=shrv[0:1, :], scale=invW)
            nc.sync.dma_start(cc_in[h].rearrange("(o e) -> o e", o=1),
                              stg[h][:])
            nc.gpsimd.collective_compute(
                "AllReduce", mybir.AluOpType.add, replica_groups=rg,
                ins=[cc_in[h]], outs=[cc_out[h]])

        def half_scale(k, h, src):
            """X = g * src  (AR-independent; fills the AllReduce shadow)."""
            for cc in range(HCH):
                c = h * HCH + cc
                nc.vector.tensor_scalar(X[c][:], src[c][:], geff[:], 0.0,
                                        alu.mult, alu.add)

        def half_update(k, h):
            """bcast s -> t = x0*s -> X += t (fused accum) -> sq / store."""
            last = k == NL - 1
            nc.sync.dma_start(
                s_sb[h][:],
                cc_out[h].rearrange("(o e) -> o e", o=1).partition_broadcast(P))
            for cc in range(HCH):
                c = h * HCH + cc
                s_ap = s_sb[h][:, cc * CH:(cc + 1) * CH]
                tt = tp.tile([P, CH], fp16, tag="tt", name="tt")
                teng = nc.gpsimd if cc == 3 else nc.vector
                teng.tensor_tensor(tt[:], X0[c][:], s_ap, alu.mult)
                acc = None if last else ts_acc[:, c:c + 1]
                nc.vector.scalar_tensor_tensor(X[c][:], X[c][:], 1.0, tt[:],
                                               alu.mult, alu.add,
                                               accum_out=acc)
                if last:
                    st = io.tile([P, CH], f32, tag="io", name="io")
                    nc.scalar.activation(st[:], X[c][:], AF.Identity,
                                         bias=Dof[:], scale=1.0)
                    nc.sync.dma_start(outT[:, c * CH:(c + 1) * CH], st[:])
                else:
                    nc.scalar.activation(fdump[:], X[c][:], AF.Square,
                                         accum_out=sq_acc[:, c:c + 1])

        for k in range(NL):
            src = X0 if k == 0 else X
            layer_params(k)
            half_dots(k, 0, src)
            half_scale(k, 0, src)    # DVE work inside AR(h0)'s shadow
            half_dots(k, 1, src)
            half_scale(k, 1, src)
            half_update(k, 0)        # AR(h1) flies while update(h0) runs
            half_update(k, 1)
            if k < NL - 1:
                finalize_stats()

    return nc


_CACHE = {}


def _get_compiled():
    if "nc" not in _CACHE:
        nc = build_nc()
        nc.compile()
        _CACHE["nc"] = nc
    return _CACHE["nc"]


def kernel(x, gamma, beta, w, b):
    from concourse.bass_utils import run_bass_kernel_spmd

    x = np.asarray(x, dtype=np.float32)
    gamma = np.asarray(gamma, dtype=np.float32)
    beta = np.asarray(beta, dtype=np.float32)
    w = np.asarray(w, dtype=np.float32)
    b = np.asarray(b, dtype=np.float32)
    B_, L_ = x.shape

    nc = _get_compiled()
    in_maps = []
    for c in range(NCORES):
        cols = slice(c * P, (c + 1) * P)
        in_maps.append({
            "xT": np.ascontiguousarray(x[:, cols].T),
            "par": np.ascontiguousarray(np.concatenate(
                [gamma[:, cols].T, beta[:, cols].T,
                 w[:, cols].T, b[:, cols].T], axis=1)),
        })
    res = run_bass_kernel_spmd(nc, in_maps, list(range(NCORES))).results
    out = np.empty((B_, L_), np.float32)
    for c in range(NCORES):
        out[:, c * P:(c + 1) * P] = res[c]["outT"].T
    return out


# revision 7
# speedup vs baseline: 1.1485x; 1.1485x over previous
"""Trainium2 Bass kernel for nn_CrossNetwork (3x [BatchNorm1d -> cross update]).

Math per layer (reference):
    mu   = mean(x, axis=0)                  # over batch B
    var  = mean((x-mu)^2, axis=0)           # biased
    xn   = (x - mu) * gamma/sqrt(var+eps) + beta
    s    = xn @ w                           # per-row dot over features L
    x'   = x0 * s[:, None] + b + xn

Sharding: L-shard (feature/model parallel). Each of the 8 cores owns 128
of the 1024 features, laid out transposed in SBUF as [128 feature
partitions, 16384 batch free]. BatchNorm stats are core-local free-dim
reductions; the only cross-core exchange is the AllReduce of the
per-core partial dot s (fp16, one per half per layer).

Restructure: xn is never materialized. With g = gamma*rsqrt(var+eps)
and c = beta - mu*g, per layer
    x_{k+1} = g_k x_k + (c_k + b_k) + x0 * s_k .
The per-partition constant is deferred into an offset D (D_0 = 0,
D_{k+1} = g_k D_k + c_k + b_k); the stored tensor follows
    X_{k+1} = g_k X_k + t_k,   t_k = x0 * s_k,   true x_k = X_k + D_k.
The dot reads raw X_k with folded weights (w g W) and each core adds
its batch-independent share sum_p w (g D + c) at PSUM drain, pre-
AllReduce (the AllReduce then sums shares to the global constant).
Stats ride accumulators: Sum(X') on the fused update
(scalar_tensor_tensor), Sum(X'^2) on the ACT Square pass;
mean_true = Sum(X')/B + D, var = Sum(X'^2)/B - (Sum(X')/B)^2.
The last layer's ACT output pass applies +D_3 and writes f32.

Bulk data is fp16 (tolerance 2e-2, measured ~1.4e-3): DVE tensor_scalar
runs 4x, tensor_tensor 2x. The all-reduced s is partition-broadcast
into SBUF by DMA (stride-0 source AP) so the t-pass stays off PSUM.
The g-scale pass (X = g*X) is hoisted into the AllReduce shadow, and a
dummy warmup AllReduce at kernel start absorbs the ~45us communicator
init that otherwise lands on layer 1's critical path.
"""

import os
import sys

import numpy as np

for _p in ("/opt/trn_rl_repo", "/root/.axon_site/_ro/trn_rl_repo"):
    if os.path.isdir(_p) and _p not in sys.path:
        sys.path.insert(0, _p)
        break

P = 128          # feature partitions per core
NCORES = 8
NL = 3
EPS = 1e-8
MAGIC = 0x5F3759DF
WSCALE = 4096.0   # keeps w (~1e-4) out of fp16 subnormal range in the lhs


def build_nc(F=16384, n_cores=NCORES, debug=False):
    """Builds + returns the Bacc module (uncompiled). F = batch per core."""
    from contextlib import ExitStack

    import concourse.bacc as bacc
    import concourse.bass_isa as bass_isa
    import concourse.mybir as mybir
    import concourse.tile as tile
    from concourse.alu_op_type import AluOpType as alu

    dt = mybir.dt
    f32 = dt.float32
    fp16 = dt.float16
    i32 = dt.int32
    AF = mybir.ActivationFunctionType
    AX = mybir.AxisListType

    CH = 2048                 # working chunk (free dim)
    NCH = F // CH             # chunks per tensor
    HALF = F // 2
    HCH = NCH // 2            # chunks per half
    invW = 1.0 / WSCALE

    nc = bacc.Bacc("TRN2", target_bir_lowering=False, debug=debug,
                   num_devices=n_cores)

    xT = nc.dram_tensor("xT", [P, F], f32, kind="ExternalInput").ap()
    par = nc.dram_tensor("par", [P, 12], f32, kind="ExternalInput").ap()
    outT = nc.dram_tensor("outT", [P, F], f32, kind="ExternalOutput").ap()
    cc_in = [nc.dram_tensor(f"cc_in{h}", [HALF], fp16).ap() for h in range(2)]
    cc_out = [nc.dram_tensor(f"cc_out{h}", [HALF], fp16,
                             addr_space="Shared").ap() for h in range(2)]
    rg = [list(range(n_cores))]

    with tile.TileContext(nc) as tc, ExitStack() as ctx:
        big = ctx.enter_context(tc.tile_pool(name="big", bufs=1))
        sm = ctx.enter_context(tc.tile_pool(name="small", bufs=1))
        io = ctx.enter_context(tc.tile_pool(name="io", bufs=4))
        tp = ctx.enter_context(tc.tile_pool(name="tmul", bufs=4))
        pdot = ctx.enter_context(tc.tile_pool(name="pdot", bufs=2, space="PSUM"))

        # persistent big tiles, chunked so Tile pipelines at chunk grain
        X0 = [big.tile([P, CH], fp16, tag=f"X0_{i}", name=f"X0_{i}")
              for i in range(NCH)]
        X = [big.tile([P, CH], fp16, tag=f"X_{i}", name=f"X_{i}")
             for i in range(NCH)]
        s_sb = [big.tile([P, HALF], fp16, tag=f"ssb{h}", name=f"ssb{h}")
                for h in range(2)]
        stg = [big.tile([1, HALF], fp16, tag=f"stg{h}", name=f"stg{h}")
               for h in range(2)]
        fdump = big.tile([P, CH], fp16, tag="fdump", name="fdump")

        par_sb = sm.tile([P, 12], f32)
        w_h = sm.tile([P, NL], fp16)
        ts_acc = sm.tile([P, NCH], f32)
        sq_acc = sm.tile([P, NCH], f32)
        geff = sm.tile([P, 1], f32)
        cbias = sm.tile([P, 1], f32)
        Dof = sm.tile([P, 1], f32)
        veps = sm.tile([P, 1], f32)
        rsq = sm.tile([P, 1], f32)
        nt1 = sm.tile([P, 1], f32)
        nt2 = sm.tile([P, 1], f32)
        shrv = sm.tile([P, 1], f32)
        tsum = sm.tile([P, 1], f32)
        ssum = sm.tile([P, 1], f32)
        mean_t = sm.tile([P, 1], f32)
        var_t = sm.tile([P, 1], f32)

        nc.sync.dma_start(par_sb[:], par[:])
        nc.vector.memset(Dof[:], 0.0)

        # ---- load: DMA f32 chunk -> fp16 convert (DVE, accum Sum x) ----
        # ----       -> Square (ACT, accum Sum x^2)                   ----
        for c in range(NCH):
            sl = slice(c * CH, (c + 1) * CH)
            ld = io.tile([P, CH], f32, tag="io", name="io")
            deng = nc.sync if c % 2 == 0 else nc.scalar
            deng.dma_start(ld[:], xT[:, sl])
            nc.vector.tensor_scalar(X0[c][:], ld[:], 1.0, 0.0,
                                    alu.mult, alu.add,
                                    accum_out=ts_acc[:, c:c + 1])
            nc.scalar.activation(fdump[:], X0[c][:], AF.Square,
                                 accum_out=sq_acc[:, c:c + 1])

        def finalize_stats():
            """mean/var of stored X from the chunk accumulators."""
            nc.vector.tensor_reduce(tsum[:], ts_acc[:], AX.X, alu.add)
            nc.vector.tensor_reduce(ssum[:], sq_acc[:], AX.X, alu.add)
            # m_st = Sum(X)/F ; mean_true = m_st + D ; var = Sum(X^2)/F - m_st^2
            nc.vector.tensor_scalar(tsum[:], tsum[:], 1.0 / F, None, alu.mult)
            nc.vector.tensor_tensor(mean_t[:], tsum[:], Dof[:], alu.add)
            nc.vector.tensor_tensor(nt1[:], tsum[:], tsum[:], alu.mult)
            nc.vector.tensor_scalar(ssum[:], ssum[:], 1.0 / F, None, alu.mult)
            nc.vector.tensor_tensor(var_t[:], ssum[:], nt1[:], alu.subtract)

        finalize_stats()

        def layer_params(k):
            # rsqrt(var+eps): quake seed + 2 Newton iterations (all DVE).
            # Order: g and the folded dot lhs first (unblocks PE dots),
            # then c/share (gates drains), then D (gates update/output).
            nc.vector.tensor_scalar(veps[:], var_t[:], EPS, None, alu.add)
            vi = veps[:].bitcast(i32)
            ri = rsq[:].bitcast(i32)
            nc.vector.tensor_scalar(ri, vi, 1, None, alu.logical_shift_right)
            nc.vector.tensor_scalar(ri, ri, -1, MAGIC, alu.mult, alu.add)
            r = rsq[:]
            for _ in range(2):
                nc.vector.tensor_tensor(nt1[:], r, r, alu.mult)
                nc.vector.tensor_tensor(nt1[:], nt1[:], veps[:], alu.mult)
                nc.vector.tensor_scalar(nt1[:], nt1[:], -0.5, 1.5,
                                        alu.mult, alu.add)
                nc.vector.tensor_tensor(r, r, nt1[:], alu.mult)
            # g = gamma * rsqrt ; folded dot lhs (w*g)*WSCALE fp16
            nc.vector.tensor_tensor(geff[:], par_sb[:, k:k + 1], r, alu.mult)
            nc.vector.tensor_tensor(nt1[:], par_sb[:, 6 + k:7 + k], geff[:],
                                    alu.mult)
            nc.vector.tensor_scalar(w_h[:, k:k + 1], nt1[:], WSCALE, None,
                                    alu.mult)
            # c = beta - mean_true * g ; share = sum_p w*(g*D + c)
            nc.vector.tensor_tensor(nt1[:], mean_t[:], geff[:], alu.mult)
            nc.vector.tensor_tensor(cbias[:], par_sb[:, 3 + k:4 + k], nt1[:],
                                    alu.subtract)
            nc.vector.tensor_tensor(nt2[:], geff[:], Dof[:], alu.mult)
            nc.vector.tensor_tensor(nt2[:], nt2[:], cbias[:], alu.add)
            nc.vector.tensor_tensor(nt2[:], nt2[:], par_sb[:, 6 + k:7 + k],
                                    alu.mult)
            nc.gpsimd.partition_all_reduce(shrv[:], nt2[:], P,
                                           bass_isa.ReduceOp.add)
            # D' = g*D + (c + b)   (after share, which uses the old D)
            nc.vector.tensor_tensor(nt1[:], cbias[:],
                                    par_sb[:, 9 + k:10 + k], alu.add)
            nc.vector.tensor_scalar(Dof[:], Dof[:], geff[:], nt1[:],
                                    alu.mult, alu.add)

        def half_dots(k, h, src):
            """dots -> drain (+share, /WSCALE, fp16) -> cc DMA -> AllReduce."""
            for cc in range(HCH):
                c = h * HCH + cc
                pd = pdot.tile([1, CH], f32, tag="pd", name="pd")
                for j in range(CH // 512):
                    rhs = src[c][:, j * 512:(j + 1) * 512]
                    nc.tensor.matmul(pd[0:1, j * 512:(j + 1) * 512],
                                     w_h[:, k:k + 1], rhs,
                                     start=True, stop=True)
                dst = stg[h][0:1, cc * CH:(cc + 1) * CH]
                if cc % 2 == 0:
                    nc.vector.tensor_scalar(dst, pd[0:1, :], invW,
                                            shrv[0:1, :], alu.mult, alu.add)
                else:
                    nc.scalar.activation(dst, pd[0:1, :], AF.Identity,
                                         bias=shrv[0:1, :], scale=invW)
            nc.sync.dma_start(cc_in[h].rearrange("(o e) -> o e", o=1),
                              stg[h][:])
            nc.gpsimd.collective_compute(
                "AllReduce", mybir.AluOpType.add, replica_groups=rg,
                ins=[cc_in[h]], outs=[cc_out[h]])

        def half_scale(k, h, src):
            """X = g * src  (AR-independent; fills the AllReduce shadow)."""
            for cc in range(HCH):
                c = h * HCH + cc
                nc.vector.tensor_scalar(X[c][:], src[c][:], geff[:], 0.0,
                                        alu.mult, alu.add)

        def half_update(k, h):
            """bcast s -> t = x0*s -> X += t -> sums (GPSIMD) / sq / store."""
            last = k == NL - 1
            sb = cc_out[h].rearrange("(o e) -> o e", o=1)
            HB = HALF // 2
            nc.sync.dma_start(s_sb[h][:, 0:HB],
                              sb[:, 0:HB].partition_broadcast(P))
            nc.scalar.dma_start(s_sb[h][:, HB:HALF],
                                sb[:, HB:HALF].partition_broadcast(P))
            for cc in range(HCH):
                c = h * HCH + cc
                s_ap = s_sb[h][:, cc * CH:(cc + 1) * CH]
                tt = tp.tile([P, CH], fp16, tag="tt", name="tt")
                nc.vector.tensor_tensor(tt[:], X0[c][:], s_ap, alu.mult)
                if last:
                    nc.vector.tensor_tensor(X[c][:], X[c][:], tt[:], alu.add)
                    st = io.tile([P, CH], f32, tag="io", name="io")
                    nc.scalar.activation(st[:], X[c][:], AF.Identity,
                                         bias=Dof[:], scale=1.0)
                    deng = nc.sync if cc % 2 == 0 else nc.scalar
                    deng.dma_start(outT[:, c * CH:(c + 1) * CH], st[:])
                else:
                    nc.vector.scalar_tensor_tensor(
                        X[c][:], X[c][:], 1.0, tt[:], alu.mult, alu.add,
                        accum_out=ts_acc[:, c:c + 1])
                    nc.scalar.activation(fdump[:], X[c][:], AF.Square,
                                         accum_out=sq_acc[:, c:c + 1])

        for k in range(NL):
            src = X0 if k == 0 else X
            layer_params(k)
            half_dots(k, 0, src)
            half_scale(k, 0, src)    # DVE work inside AR(h0)'s shadow
            half_dots(k, 1, src)
            half_scale(k, 1, src)
            half_update(k, 0)        # AR(h1) flies while update(h0) runs
            half_update(k, 1)
            if k < NL - 1:
                finalize_stats()

    return nc


_CACHE = {}


def _get_compiled():
    if "nc" not in _CACHE:
        nc = build_nc()
        nc.compile()
        _CACHE["nc"] = nc
    return _CACHE["nc"]


def kernel(x, gamma, beta, w, b):
    from concourse.bass_utils import run_bass_kernel_spmd

    x = np.asarray(x, dtype=np.float32)
    gamma = np.asarray(gamma, dtype=np.float32)
    beta = np.asarray(beta, dtype=np.float32)
    w = np.asarray(w, dtype=np.float32)
    b = np.asarray(b, dtype=np.float32)
    B_, L_ = x.shape

    nc = _get_compiled()
    in_maps = []
    for c in range(NCORES):
        cols = slice(c * P, (c + 1) * P)
        in_maps.append({
            "xT": np.ascontiguousarray(x[:, cols].T),
            "par": np.ascontiguousarray(np.concatenate(
                [gamma[:, cols].T, beta[:, cols].T,
                 w[:, cols].T, b[:, cols].T], axis=1)),
        })
    res = run_bass_kernel_spmd(nc, in_maps, list(range(NCORES))).results
    out = np.empty((B_, L_), np.float32)
    for c in range(NCORES):
        out[:, c * P:(c + 1) * P] = res[c]["outT"].T
    return out
